# revision 3
# baseline (speedup 1.0000x reference)
"""LocalizationAttacks kernel for 8 Trainium2 NeuronCores.

Data-parallel over the batch dim: each of the 8 cores processes 4 of the 32
batch items. Per-segment attack decisions (tiny [B, 300] masks) are
precomputed on the host and shipped as per-partition scalars; the 300 MB of
audio streaming (2 input streams, 3 output streams) runs on-device and is
fabric-bound at ~430 GB/s per core (measured: 2 HWDGE queues plateau at
425-438 GB/s; adding the gpsimd dynamic queue LOWERS aggregate to ~365).
Floor = 38.4 MB / 430 GB/s ~= 89 us + ~7.2 us fixed preamble + drain.

Schedule: keep both HWDGE queues busy from ~8 us to ~97 us, with compute
spread across three engines so no queue ever waits on a convoy:
  - SYNC queue: all 12 input loads in tile order, then att/uo stores of
    tiles 0 and 5 (t0's data is computed early; t5 is small and ready
    right after the last load lands).
  - ACT queue: mask load first, then ground_truth stores (per-slice; gt
    depends only on the mask so these stream from ~9 us), then att/uo
    stores of tiles 1-4.
  - GPSIMD computes gt (ones * mask_col) and uo (og * zm_col) broadcasts.
  - DVE computes only att (mul + scalar_tensor_tensor, ~35 us total).
Engine coupling is kept loose (wm/og bufs=3) so load issues never convoy
behind compute, which convoys behind store completions.
"""

import numpy as np

import concourse.bacc as bacc
import concourse.bass as bass
import concourse.mybir as mybir
from concourse.bass_utils import run_bass_kernel_spmd
from concourse.tile import TileContext

# Problem shape (hardcoded per contract)
B, C, T = 32, 1, 480000
SEG = 1600
S = T // SEG              # 300 segments per item
N_CORES = 8
B_LOC = B // N_CORES      # 4 items per core
N_SEGS = B_LOC * S        # 1200 segments per core
P = 128

# (partitions, segments-per-partition-row) per tile; rows sum to N_SEGS
PLAN = [(128, 2), (128, 2), (128, 2), (62, 2), (128, 2), (26, 2)]
assert sum(p * k for p, k in PLAN) == N_SEGS
N_MASK_COLS = 3 * sum(k for _, k in PLAN)
SYNC_TILES = (0, 5)  # att/uo of these tiles stored on the sync queue

F32 = mybir.dt.float32


def _build_nc() -> bass.Bass:
    nc = bacc.Bacc()
    wm = nc.dram_tensor("wm", [N_SEGS * SEG], F32, kind="ExternalInput")
    og = nc.dram_tensor("og", [N_SEGS * SEG], F32, kind="ExternalInput")
    mk = nc.dram_tensor("mk", [P, N_MASK_COLS], F32, kind="ExternalInput")
    att = nc.dram_tensor("att", [N_SEGS * SEG], F32, kind="ExternalOutput")
    gt = nc.dram_tensor("gt", [N_SEGS * SEG], F32, kind="ExternalOutput")
    uo = nc.dram_tensor("uo", [N_SEGS * SEG], F32, kind="ExternalOutput")

    mult = mybir.AluOpType.mult
    add = mybir.AluOpType.add

    def view(t, e0, p, k):
        return t[e0 : e0 + p * k * SEG].rearrange("(p f) -> p f", p=p)

    # per-tile (dram offset, mask column block offset)
    offs = []
    e0 = off = 0
    for p, k in PLAN:
        offs.append((e0, off))
        e0 += p * k * SEG
        off += k

    with TileContext(nc) as tc:
        with tc.tile_pool(name="io", bufs=2) as pool:
            pad = [P, 2 * SEG]
            m_all = pool.tile([P, N_MASK_COLS], F32, tag="m", bufs=1)
            nc.scalar.dma_start(out=m_all[:], in_=mk[:, :])
            ones_t = pool.tile([P, SEG], F32, tag="ones", bufs=1)
            nc.gpsimd.memset(ones_t[:], 1.0)

            # all loads on the sync queue, tile order
            in_tiles = []
            for i, (p, k) in enumerate(PLAN):
                e0 = offs[i][0]
                wm_t = pool.tile([p, k * SEG], F32, tag="wm", bufs=3, padded_shape=pad)
                og_t = pool.tile([p, k * SEG], F32, tag="og", bufs=3, padded_shape=pad)
                nc.sync.dma_start(out=wm_t[:], in_=view(wm, e0, p, k))
                nc.sync.dma_start(out=og_t[:], in_=view(og, e0, p, k))
                in_tiles.append((wm_t, og_t))

            # gt: gpsimd computes each [p, SEG] slice, ACT stores it
            for i, (p, k) in enumerate(PLAN):
                e0, off = offs[i]
                for j in range(k):
                    c = 3 * (off + j)
                    gts = pool.tile([p, SEG], F32, tag="gt", bufs=2,
                                    padded_shape=[P, SEG])
                    nc.gpsimd.tensor_scalar_mul(
                        gts[:], ones_t[:p, :], m_all[:p, c : c + 1]
                    )
                    gv = view(gt, e0, p, k)[:, j * SEG : (j + 1) * SEG]
                    nc.scalar.dma_start(out=gv, in_=gts[:])

            # att on DVE, tile order
            at_tiles = {}
            for i, (p, k) in enumerate(PLAN):
                off = offs[i][1]
                tag, nb = ("as", 1) if i in SYNC_TILES else ("aa", 3)
                at_t = pool.tile([p, k * SEG], F32, tag=tag, bufs=nb, padded_shape=pad)
                at_tiles[i] = at_t
                wm_t, og_t = in_tiles[i]
                for j in range(k):
                    sl = slice(j * SEG, (j + 1) * SEG)
                    c = 3 * (off + j)
                    s_am = m_all[:p, c + 0 : c + 1]  # 1 - attack
                    s_rm = m_all[:p, c + 1 : c + 2]  # revert
                    nc.vector.tensor_scalar_mul(at_t[:, sl], og_t[:, sl], s_rm)
                    nc.vector.scalar_tensor_tensor(
                        at_t[:, sl], wm_t[:, sl], s_am, at_t[:, sl], mult, add
                    )

            # uo on gpsimd, tile order
            uo_tiles = {}
            for i, (p, k) in enumerate(PLAN):
                off = offs[i][1]
                tag, nb = ("us", 2) if i in SYNC_TILES else ("ua", 2)
                uo_t = pool.tile([p, k * SEG], F32, tag=tag, bufs=nb, padded_shape=pad)
                uo_tiles[i] = uo_t
                og_t = in_tiles[i][1]
                for j in range(k):
                    sl = slice(j * SEG, (j + 1) * SEG)
                    c = 3 * (off + j)
                    s_zm = m_all[:p, c + 2 : c + 3]  # 1 - zero
                    nc.gpsimd.tensor_scalar_mul(uo_t[:, sl], og_t[:, sl], s_zm)

            def emit_store(i, ring):
                p, k = PLAN[i]
                e0 = offs[i][0]
                ring.dma_start(out=view(att, e0, p, k), in_=at_tiles[i][:])
                ring.dma_start(out=view(uo, e0, p, k), in_=uo_tiles[i][:])

            emit_store(0, nc.sync)
            for i in (1, 2, 3, 4):
                emit_store(i, nc.scalar)
            emit_store(5, nc.sync)
    nc.compile()
    return nc


_NC_CACHE: bass.Bass | None = None


def _pack_masks(oma_rows, rm_rows, omz_rows):
    """Per-core segment masks [N_SEGS] -> one [P, N_MASK_COLS] tile."""
    m_all = np.zeros((P, N_MASK_COLS), np.float32)
    r0 = 0
    off = 0
    for p, k in PLAN:
        for j in range(k):
            c = 3 * (off + j)
            # partition q, slice j holds segment r0 + q*k + j
            m_all[:p, c + 0] = oma_rows[r0 + j : r0 + p * k : k]
            m_all[:p, c + 1] = rm_rows[r0 + j : r0 + p * k : k]
            m_all[:p, c + 2] = omz_rows[r0 + j : r0 + p * k : k]
        r0 += p * k
        off += k
    return m_all


def _prepare_in_maps(original, watermarked, seg_starts, revert_flags):
    original = np.ascontiguousarray(np.asarray(original), dtype=np.float32)
    watermarked = np.ascontiguousarray(np.asarray(watermarked), dtype=np.float32)
    seg_starts = np.asarray(seg_starts)
    revert_flags = np.asarray(revert_flags)

    # Host-side segment masks, [B, 300] each (tiny).
    attack = np.zeros((B, S), np.float32)
    attack[np.arange(B)[:, None], seg_starts] = 1.0
    rf = revert_flags.astype(np.float32)
    one_minus_am = 1.0 - attack
    rm = attack * rf
    one_minus_zm = 1.0 - attack * (1.0 - rf)

    in_maps = []
    for c in range(N_CORES):
        sl = slice(c * B_LOC, (c + 1) * B_LOC)
        in_maps.append(
            {
                "wm": watermarked[sl].reshape(-1),
                "og": original[sl].reshape(-1),
                "mk": _pack_masks(
                    one_minus_am[sl].reshape(-1),
                    rm[sl].reshape(-1),
                    one_minus_zm[sl].reshape(-1),
                ),
            }
        )
    return in_maps


def _gather(results):
    def cat(name):
        return np.concatenate(
            [results[c][name].reshape(B_LOC, C, T) for c in range(N_CORES)], axis=0
        )

    return cat("att"), cat("gt"), cat("uo")


def _run(inputs: dict, **run_kwargs):
    global _NC_CACHE
    if _NC_CACHE is None:
        _NC_CACHE = _build_nc()
    in_maps = _prepare_in_maps(**inputs)
    res = run_bass_kernel_spmd(
        _NC_CACHE, in_maps, core_ids=list(range(N_CORES)), **run_kwargs
    )
    return res, _gather(res.results)


def kernel(original, watermarked, seg_starts, revert_flags):
    _, outs = _run(
        dict(
            original=original,
            watermarked=watermarked,
            seg_starts=seg_starts,
            revert_flags=revert_flags,
        )
    )
    return outs


# revision 4
# speedup vs baseline: 3.0121x; 3.0121x over previous
"""LocalizationAttacks kernel for 8 Trainium2 NeuronCores.

Data-parallel over the batch dim: each of the 8 cores processes 4 of the 32
batch items. Per-segment attack decisions (tiny [B, 300] masks) are
precomputed on the host and shipped as per-partition scalars; the 300 MB of
audio streaming (2 input streams, 3 output streams) runs on-device and is
fabric-bound at ~430 GB/s per core (measured: 2 HWDGE queues plateau at
425-438 GB/s; adding the gpsimd dynamic queue LOWERS aggregate to ~365).
Floor = 38.4 MB / 430 GB/s ~= 89 us + ~7.2 us fixed preamble + drain.

Schedule: keep both HWDGE queues busy from ~8 us to ~97 us, with compute
spread across three engines so no queue ever waits on a convoy:
  - SYNC queue: all 12 input loads in tile order, then att/uo stores of
    tiles 0 and 5 (t0's data is computed early; t5 is small and ready
    right after the last load lands).
  - ACT queue: mask load first, then ground_truth stores (per-slice; gt
    depends only on the mask so these stream from ~9 us), then att/uo
    stores of tiles 1-4.
  - GPSIMD computes gt (ones * mask_col) and uo (og * zm_col) broadcasts.
  - DVE computes only att (mul + scalar_tensor_tensor, ~35 us total).
Engine coupling is kept loose (wm/og bufs=3) so load issues never convoy
behind compute, which convoys behind store completions.
"""

import numpy as np

import concourse.bacc as bacc
import concourse.bass as bass
import concourse.mybir as mybir
from concourse.bass_utils import run_bass_kernel_spmd
from concourse.tile import TileContext

# Problem shape (hardcoded per contract)
B, C, T = 32, 1, 480000
SEG = 1600
S = T // SEG              # 300 segments per item
N_CORES = 8
B_LOC = B // N_CORES      # 4 items per core
N_SEGS = B_LOC * S        # 1200 segments per core
P = 128

# (partitions, segments-per-partition-row) per tile; rows sum to N_SEGS
PLAN = [(128, 2), (128, 2), (128, 2), (62, 2), (128, 2), (26, 2)]
assert sum(p * k for p, k in PLAN) == N_SEGS
N_MASK_COLS = 3 * sum(k for _, k in PLAN)
SYNC_TILES = (0, 5)  # att/uo of these tiles stored on the sync queue

F32 = mybir.dt.float32


def _build_nc() -> bass.Bass:
    nc = bacc.Bacc()
    wm = nc.dram_tensor("wm", [N_SEGS * SEG], F32, kind="ExternalInput")
    og = nc.dram_tensor("og", [N_SEGS * SEG], F32, kind="ExternalInput")
    mk = nc.dram_tensor("mk", [P, N_MASK_COLS], F32, kind="ExternalInput")
    att = nc.dram_tensor("att", [N_SEGS * SEG], F32, kind="ExternalOutput")
    gt = nc.dram_tensor("gt", [N_SEGS * SEG], F32, kind="ExternalOutput")
    uo = nc.dram_tensor("uo", [N_SEGS * SEG], F32, kind="ExternalOutput")

    mult = mybir.AluOpType.mult
    add = mybir.AluOpType.add

    def view(t, e0, p, k):
        return t[e0 : e0 + p * k * SEG].rearrange("(p f) -> p f", p=p)

    # per-tile (dram offset, mask column block offset)
    offs = []
    e0 = off = 0
    for p, k in PLAN:
        offs.append((e0, off))
        e0 += p * k * SEG
        off += k

    with TileContext(nc) as tc:
        with tc.tile_pool(name="io", bufs=2) as pool:
            pad = [P, 2 * SEG]
            m_all = pool.tile([P, N_MASK_COLS], F32, tag="m", bufs=1)
            nc.scalar.dma_start(out=m_all[:], in_=mk[:, :])
            ones_t = pool.tile([P, SEG], F32, tag="ones", bufs=1)
            nc.gpsimd.memset(ones_t[:], 1.0)

            # all loads on the sync queue, tile order
            in_tiles = []
            for i, (p, k) in enumerate(PLAN):
                e0 = offs[i][0]
                wm_t = pool.tile([p, k * SEG], F32, tag="wm", bufs=3, padded_shape=pad)
                og_t = pool.tile([p, k * SEG], F32, tag="og", bufs=3, padded_shape=pad)
                nc.sync.dma_start(out=wm_t[:], in_=view(wm, e0, p, k))
                nc.sync.dma_start(out=og_t[:], in_=view(og, e0, p, k))
                in_tiles.append((wm_t, og_t))

            # gt: the ACT engine computes each [p, SEG] slice (ones * mask
            # col) and immediately issues its store -- the whole gt stream
            # is self-contained on ACT, gated only by the mask load.
            for i, (p, k) in enumerate(PLAN):
                e0, off = offs[i]
                for j in range(k):
                    c = 3 * (off + j)
                    gts = pool.tile([p, SEG], F32, tag="gt", bufs=2,
                                    padded_shape=[P, SEG])
                    nc.scalar.mul(gts[:], ones_t[:p, :], m_all[:p, c : c + 1])
                    gv = view(gt, e0, p, k)[:, j * SEG : (j + 1) * SEG]
                    nc.scalar.dma_start(out=gv, in_=gts[:])

            # att on DVE, tile order
            at_tiles = {}
            for i, (p, k) in enumerate(PLAN):
                off = offs[i][1]
                tag, nb = ("as", 1) if i in SYNC_TILES else ("aa", 3)
                at_t = pool.tile([p, k * SEG], F32, tag=tag, bufs=nb, padded_shape=pad)
                at_tiles[i] = at_t
                wm_t, og_t = in_tiles[i]
                for j in range(k):
                    sl = slice(j * SEG, (j + 1) * SEG)
                    c = 3 * (off + j)
                    s_am = m_all[:p, c + 0 : c + 1]  # 1 - attack
                    s_rm = m_all[:p, c + 1 : c + 2]  # revert
                    nc.vector.tensor_scalar_mul(at_t[:, sl], og_t[:, sl], s_rm)
                    nc.vector.scalar_tensor_tensor(
                        at_t[:, sl], wm_t[:, sl], s_am, at_t[:, sl], mult, add
                    )

            # uo computed on ACT (og * zm col), stored by its ring; computes
            # are placed in ACT's stream right before the consuming store so
            # waits never block an otherwise-ready store issue.
            uo_tiles = {}

            def compute_uo(i):
                p, k = PLAN[i]
                off = offs[i][1]
                tag = "us" if i in SYNC_TILES else "ua"
                uo_t = pool.tile([p, k * SEG], F32, tag=tag, bufs=2, padded_shape=pad)
                uo_tiles[i] = uo_t
                og_t = in_tiles[i][1]
                for j in range(k):
                    sl = slice(j * SEG, (j + 1) * SEG)
                    c = 3 * (off + j)
                    nc.scalar.mul(uo_t[:, sl], og_t[:, sl], m_all[:p, c + 2 : c + 3])

            def emit_store(i, ring):
                p, k = PLAN[i]
                e0 = offs[i][0]
                ring.dma_start(out=view(att, e0, p, k), in_=at_tiles[i][:])
                ring.dma_start(out=view(uo, e0, p, k), in_=uo_tiles[i][:])

            compute_uo(0)
            emit_store(0, nc.sync)
            for i in (1, 2, 3, 4):
                compute_uo(i)
                emit_store(i, nc.scalar)
            compute_uo(5)
            emit_store(5, nc.sync)
    nc.compile()
    return nc


_NC_CACHE: bass.Bass | None = None


def _pack_masks(oma_rows, rm_rows, omz_rows):
    """Per-core segment masks [N_SEGS] -> one [P, N_MASK_COLS] tile."""
    m_all = np.zeros((P, N_MASK_COLS), np.float32)
    r0 = 0
    off = 0
    for p, k in PLAN:
        for j in range(k):
            c = 3 * (off + j)
            # partition q, slice j holds segment r0 + q*k + j
            m_all[:p, c + 0] = oma_rows[r0 + j : r0 + p * k : k]
            m_all[:p, c + 1] = rm_rows[r0 + j : r0 + p * k : k]
            m_all[:p, c + 2] = omz_rows[r0 + j : r0 + p * k : k]
        r0 += p * k
        off += k
    return m_all


def _prepare_in_maps(original, watermarked, seg_starts, revert_flags):
    original = np.ascontiguousarray(np.asarray(original), dtype=np.float32)
    watermarked = np.ascontiguousarray(np.asarray(watermarked), dtype=np.float32)
    seg_starts = np.asarray(seg_starts)
    revert_flags = np.asarray(revert_flags)

    # Host-side segment masks, [B, 300] each (tiny).
    attack = np.zeros((B, S), np.float32)
    attack[np.arange(B)[:, None], seg_starts] = 1.0
    rf = revert_flags.astype(np.float32)
    one_minus_am = 1.0 - attack
    rm = attack * rf
    one_minus_zm = 1.0 - attack * (1.0 - rf)

    in_maps = []
    for c in range(N_CORES):
        sl = slice(c * B_LOC, (c + 1) * B_LOC)
        in_maps.append(
            {
                "wm": watermarked[sl].reshape(-1),
                "og": original[sl].reshape(-1),
                "mk": _pack_masks(
                    one_minus_am[sl].reshape(-1),
                    rm[sl].reshape(-1),
                    one_minus_zm[sl].reshape(-1),
                ),
            }
        )
    return in_maps


def _gather(results):
    def cat(name):
        return np.concatenate(
            [results[c][name].reshape(B_LOC, C, T) for c in range(N_CORES)], axis=0
        )

    return cat("att"), cat("gt"), cat("uo")


def _run(inputs: dict, **run_kwargs):
    global _NC_CACHE
    if _NC_CACHE is None:
        _NC_CACHE = _build_nc()
    in_maps = _prepare_in_maps(**inputs)
    res = run_bass_kernel_spmd(
        _NC_CACHE, in_maps, core_ids=list(range(N_CORES)), **run_kwargs
    )
    return res, _gather(res.results)


def kernel(original, watermarked, seg_starts, revert_flags):
    _, outs = _run(
        dict(
            original=original,
            watermarked=watermarked,
            seg_starts=seg_starts,
            revert_flags=revert_flags,
        )
    )
    return outs


# revision 5
# speedup vs baseline: 4.8629x; 1.6145x over previous
"""LocalizationAttacks kernel for 8 Trainium2 NeuronCores.

Data-parallel over the batch dim: each of the 8 cores processes 4 of the 32
batch items. Per-segment attack decisions (tiny [B, 300] masks) are
precomputed on the host and shipped as per-partition scalars; the 300 MB of
audio streaming (2 input streams, 3 output streams) runs on-device and is
fabric-bound at ~430 GB/s per core (measured: the 2 HWDGE queues together
plateau at 425-438 GB/s; adding the gpsimd dynamic queue LOWERS the
aggregate; gpsimd/ACT compute is 10-20x slower than DVE so all elementwise
math stays on DVE and both HWDGE engines stay pure DMA issuers).
Floor = 38.4 MB / 430 GB/s ~= 89 us + ~7.2 us fixed preamble + drain.

Schedule (vs the naive version): keep both queues streaming end-to-end:
  - SYNC queue: all 12 input loads in tile order, then att/uo stores of
    tiles 0 and 5. Tile 0 is computed early so sync's tail stores are
    ready the moment its loads finish (no dependence on the last loads);
    tile 5's data is ready right after the last load lands.
  - ACT queue: mask load first (gates nothing but gt), then ground_truth
    stores per-slice (gt depends only on the mask, so ACT streams stores
    from ~10 us), then att/uo stores of tiles 1-4 in readiness order.
  - DVE: gt slices are store-paced (gt bufs=3), so t0-t3 att/uo compute is
    woven between gt slices to fill the pacing stalls; t4/t5 run when
    their loads land. att/uo tiles destined for sync (t0/t5) use separate
    tags so DVE never waits on sync's late tail stores for buffers that
    ACT-bound tiles need.
Byte balance: sync 19.25 MB vs ACT 19.16 MB (ACT starts ~2 us later).
"""

import numpy as np

import concourse.bacc as bacc
import concourse.bass as bass
import concourse.mybir as mybir
from concourse.bass_utils import run_bass_kernel_spmd
from concourse.tile import TileContext

# Problem shape (hardcoded per contract)
B, C, T = 32, 1, 480000
SEG = 1600
S = T // SEG              # 300 segments per item
N_CORES = 8
B_LOC = B // N_CORES      # 4 items per core
N_SEGS = B_LOC * S        # 1200 segments per core
P = 128

# (partitions, segments-per-partition-row) per tile; rows sum to N_SEGS.
# 10 total column-slices is the minimum (ceil(1200/128)) -- DVE op count
# scales with slice count, not bytes.
PLAN = [(128, 1), (128, 1), (128, 2), (128, 2), (128, 2), (88, 2)]
assert sum(p * k for p, k in PLAN) == N_SEGS
N_MASK_COLS = 3 * sum(k for _, k in PLAN)
SYNC_TILES = (0, 5)  # att/uo of these tiles stored on the sync queue

F32 = mybir.dt.float32


def _build_nc() -> bass.Bass:
    nc = bacc.Bacc()
    wm = nc.dram_tensor("wm", [N_SEGS * SEG], F32, kind="ExternalInput")
    og = nc.dram_tensor("og", [N_SEGS * SEG], F32, kind="ExternalInput")
    mk = nc.dram_tensor("mk", [P, N_MASK_COLS], F32, kind="ExternalInput")
    att = nc.dram_tensor("att", [N_SEGS * SEG], F32, kind="ExternalOutput")
    gt = nc.dram_tensor("gt", [N_SEGS * SEG], F32, kind="ExternalOutput")
    uo = nc.dram_tensor("uo", [N_SEGS * SEG], F32, kind="ExternalOutput")

    mult = mybir.AluOpType.mult
    add = mybir.AluOpType.add

    def view(t, e0, p, k):
        return t[e0 : e0 + p * k * SEG].rearrange("(p f) -> p f", p=p)

    # per-tile (dram offset, mask column block offset)
    offs = []
    e0 = off = 0
    for p, k in PLAN:
        offs.append((e0, off))
        e0 += p * k * SEG
        off += k

    with TileContext(nc) as tc:
        with tc.tile_pool(name="io", bufs=2) as pool:
            pad = [P, 2 * SEG]
            m_all = pool.tile([P, N_MASK_COLS], F32, tag="m", bufs=1)
            nc.scalar.dma_start(out=m_all[:], in_=mk[:, :])
            ones_t = pool.tile([P, SEG], F32, tag="ones", bufs=1)
            nc.gpsimd.memset(ones_t[:], 1.0)

            # all loads on the sync queue, tile order
            in_tiles = []
            for i, (p, k) in enumerate(PLAN):
                e0 = offs[i][0]
                wm_t = pool.tile([p, k * SEG], F32, tag="wm", bufs=3, padded_shape=pad)
                og_t = pool.tile([p, k * SEG], F32, tag="og", bufs=3, padded_shape=pad)
                nc.sync.dma_start(out=wm_t[:], in_=view(wm, e0, p, k))
                nc.sync.dma_start(out=og_t[:], in_=view(og, e0, p, k))
                in_tiles.append((wm_t, og_t))

            at_tiles = {}
            uo_tiles = {}

            def emit_gt(i, j):
                p, k = PLAN[i]
                e0, off = offs[i]
                c = 3 * (off + j)
                gts = pool.tile([p, SEG], F32, tag="gt", bufs=3,
                                padded_shape=[P, SEG])
                nc.vector.tensor_scalar_mul(gts[:], ones_t[:p, :],
                                            m_all[:p, c : c + 1])
                gv = view(gt, e0, p, k)[:, j * SEG : (j + 1) * SEG]
                nc.scalar.dma_start(out=gv, in_=gts[:])

            def compute_tile(i):
                p, k = PLAN[i]
                off = offs[i][1]
                a_tag, u_tag, nb = (
                    ("as", "us", 1) if i in SYNC_TILES else ("aa", "ua", 3)
                )
                at_t = pool.tile([p, k * SEG], F32, tag=a_tag, bufs=nb,
                                 padded_shape=pad)
                uo_t = pool.tile([p, k * SEG], F32, tag=u_tag, bufs=nb,
                                 padded_shape=pad)
                at_tiles[i], uo_tiles[i] = at_t, uo_t
                wm_t, og_t = in_tiles[i]
                for j in range(k):
                    sl = slice(j * SEG, (j + 1) * SEG)
                    c = 3 * (off + j)
                    s_am = m_all[:p, c + 0 : c + 1]  # 1 - attack
                    s_rm = m_all[:p, c + 1 : c + 2]  # revert
                    s_zm = m_all[:p, c + 2 : c + 3]  # 1 - zero
                    nc.vector.tensor_scalar_mul(at_t[:, sl], og_t[:, sl], s_rm)
                    nc.vector.scalar_tensor_tensor(
                        at_t[:, sl], wm_t[:, sl], s_am, at_t[:, sl], mult, add
                    )
                    nc.vector.tensor_scalar_mul(uo_t[:, sl], og_t[:, sl], s_zm)

            def emit_store(i, ring):
                p, k = PLAN[i]
                e0 = offs[i][0]
                ring.dma_start(out=view(att, e0, p, k), in_=at_tiles[i][:])
                ring.dma_start(out=view(uo, e0, p, k), in_=uo_tiles[i][:])

            # DVE weave: gt slices (ACT-store-paced) with t0-t3 compute
            # filling the pacing gaps. ACT queue order stays mask, gt x10,
            # att/uo t1..t4; sync gets att0/uo0 then att5/uo5 at its tail.
            emit_gt(0, 0)            # gt slices 0-2
            emit_gt(1, 0)
            emit_gt(2, 0)
            compute_tile(0)
            emit_store(0, nc.sync)
            emit_gt(2, 1)            # gt slice 3
            compute_tile(1)
            emit_gt(3, 0)            # gt slices 4-5
            emit_gt(3, 1)
            compute_tile(2)
            emit_gt(4, 0)            # gt slices 6-7
            emit_gt(4, 1)
            compute_tile(3)
            emit_gt(5, 0)            # gt slices 8-9
            emit_gt(5, 1)
            for i in (1, 2, 3):
                emit_store(i, nc.scalar)
            compute_tile(4)
            emit_store(4, nc.scalar)
            compute_tile(5)
            emit_store(5, nc.sync)
    nc.compile()
    return nc


_NC_CACHE: bass.Bass | None = None


def _pack_masks(oma_rows, rm_rows, omz_rows):
    """Per-core segment masks [N_SEGS] -> one [P, N_MASK_COLS] tile."""
    m_all = np.zeros((P, N_MASK_COLS), np.float32)
    r0 = 0
    off = 0
    for p, k in PLAN:
        for j in range(k):
            c = 3 * (off + j)
            # partition q, slice j holds segment r0 + q*k + j
            m_all[:p, c + 0] = oma_rows[r0 + j : r0 + p * k : k]
            m_all[:p, c + 1] = rm_rows[r0 + j : r0 + p * k : k]
            m_all[:p, c + 2] = omz_rows[r0 + j : r0 + p * k : k]
        r0 += p * k
        off += k
    return m_all


def _prepare_in_maps(original, watermarked, seg_starts, revert_flags):
    original = np.ascontiguousarray(np.asarray(original), dtype=np.float32)
    watermarked = np.ascontiguousarray(np.asarray(watermarked), dtype=np.float32)
    seg_starts = np.asarray(seg_starts)
    revert_flags = np.asarray(revert_flags)

    # Host-side segment masks, [B, 300] each (tiny).
    attack = np.zeros((B, S), np.float32)
    attack[np.arange(B)[:, None], seg_starts] = 1.0
    rf = revert_flags.astype(np.float32)
    one_minus_am = 1.0 - attack
    rm = attack * rf
    one_minus_zm = 1.0 - attack * (1.0 - rf)

    in_maps = []
    for c in range(N_CORES):
        sl = slice(c * B_LOC, (c + 1) * B_LOC)
        in_maps.append(
            {
                "wm": watermarked[sl].reshape(-1),
                "og": original[sl].reshape(-1),
                "mk": _pack_masks(
                    one_minus_am[sl].reshape(-1),
                    rm[sl].reshape(-1),
                    one_minus_zm[sl].reshape(-1),
                ),
            }
        )
    return in_maps


def _gather(results):
    def cat(name):
        return np.concatenate(
            [results[c][name].reshape(B_LOC, C, T) for c in range(N_CORES)], axis=0
        )

    return cat("att"), cat("gt"), cat("uo")


def _run(inputs: dict, **run_kwargs):
    global _NC_CACHE
    if _NC_CACHE is None:
        _NC_CACHE = _build_nc()
    in_maps = _prepare_in_maps(**inputs)
    res = run_bass_kernel_spmd(
        _NC_CACHE, in_maps, core_ids=list(range(N_CORES)), **run_kwargs
    )
    return res, _gather(res.results)


def kernel(original, watermarked, seg_starts, revert_flags):
    _, outs = _run(
        dict(
            original=original,
            watermarked=watermarked,
            seg_starts=seg_starts,
            revert_flags=revert_flags,
        )
    )
    return outs


# revision 6
# speedup vs baseline: 5.1516x; 1.0594x over previous
"""LocalizationAttacks kernel for 8 Trainium2 NeuronCores.

Data-parallel over the batch dim: each of the 8 cores processes 4 of the 32
batch items. Per-segment attack decisions (tiny [B, 300] masks) are
precomputed on the host and shipped as per-partition scalars; the 300 MB of
audio streaming (2 input streams, 3 output streams) runs on-device and is
fabric-bound at ~430 GB/s per core (measured: the 2 HWDGE queues together
plateau at 425-438 GB/s; adding the gpsimd dynamic queue LOWERS the
aggregate; gpsimd/ACT compute is 10-20x slower than DVE so all elementwise
math stays on DVE and both HWDGE engines stay pure DMA issuers).
Floor = 38.4 MB / 430 GB/s ~= 89 us + ~7.2 us fixed preamble + drain.

Schedule (vs the naive version): keep both queues streaming end-to-end:
  - SYNC queue: all 12 input loads in tile order, then att/uo stores of
    tiles 0 and 5. Tile 0 is computed early so sync's tail stores are
    ready the moment its loads finish (no dependence on the last loads);
    tile 5's data is ready right after the last load lands.
  - ACT queue: mask load first (gates nothing but gt), then ground_truth
    stores per-slice (gt depends only on the mask, so ACT streams stores
    from ~10 us), then att/uo stores of tiles 1-4 in readiness order.
  - DVE: gt slices are store-paced (gt bufs=3), so t0-t3 att/uo compute is
    woven between gt slices to fill the pacing stalls; t4/t5 run when
    their loads land. att/uo tiles destined for sync (t0/t5) use separate
    tags so DVE never waits on sync's late tail stores for buffers that
    ACT-bound tiles need.
Byte balance: sync 19.25 MB vs ACT 19.16 MB (ACT starts ~2 us later).
"""

import numpy as np

import concourse.bacc as bacc
import concourse.bass as bass
import concourse.mybir as mybir
from concourse.bass_utils import run_bass_kernel_spmd
from concourse.tile import TileContext

# Problem shape (hardcoded per contract)
B, C, T = 32, 1, 480000
SEG = 1600
S = T // SEG              # 300 segments per item
N_CORES = 8
B_LOC = B // N_CORES      # 4 items per core
N_SEGS = B_LOC * S        # 1200 segments per core
P = 128

# (partitions, segments-per-partition-row) per tile; rows sum to N_SEGS.
# 10 total column-slices is the minimum (ceil(1200/128)) -- DVE op count
# scales with slice count, not bytes.
PLAN = [(128, 1), (128, 1), (128, 2), (128, 2), (128, 2), (88, 2)]
assert sum(p * k for p, k in PLAN) == N_SEGS
N_MASK_COLS = 3 * sum(k for _, k in PLAN)
SYNC_TILES = (0, 5)  # att/uo of these tiles stored on the sync queue

F32 = mybir.dt.float32


def _build_nc() -> bass.Bass:
    nc = bacc.Bacc()
    wm = nc.dram_tensor("wm", [N_SEGS * SEG], F32, kind="ExternalInput")
    og = nc.dram_tensor("og", [N_SEGS * SEG], F32, kind="ExternalInput")
    mk = nc.dram_tensor("mk", [P, N_MASK_COLS], F32, kind="ExternalInput")
    att = nc.dram_tensor("att", [N_SEGS * SEG], F32, kind="ExternalOutput")
    gt = nc.dram_tensor("gt", [N_SEGS * SEG], F32, kind="ExternalOutput")
    uo = nc.dram_tensor("uo", [N_SEGS * SEG], F32, kind="ExternalOutput")

    mult = mybir.AluOpType.mult
    add = mybir.AluOpType.add

    def view(t, e0, p, k):
        return t[e0 : e0 + p * k * SEG].rearrange("(p f) -> p f", p=p)

    # per-tile (dram offset, mask column block offset)
    offs = []
    e0 = off = 0
    for p, k in PLAN:
        offs.append((e0, off))
        e0 += p * k * SEG
        off += k

    with TileContext(nc) as tc:
        with tc.tile_pool(name="io", bufs=2) as pool:
            pad = [P, 2 * SEG]
            m_all = pool.tile([P, N_MASK_COLS], F32, tag="m", bufs=1)
            ones_t = pool.tile([P, SEG], F32, tag="ones", bufs=1)
            in_tiles = []
            for i, (p, k) in enumerate(PLAN):
                wm_t = pool.tile([p, k * SEG], F32, tag="wm", bufs=3, padded_shape=pad)
                og_t = pool.tile([p, k * SEG], F32, tag="og", bufs=3, padded_shape=pad)
                in_tiles.append((wm_t, og_t))
            at_tiles = {}
            uo_tiles = {}

            def load(i):
                p, k = PLAN[i]
                e0 = offs[i][0]
                wm_t, og_t = in_tiles[i]
                nc.sync.dma_start(out=wm_t[:], in_=view(wm, e0, p, k))
                nc.sync.dma_start(out=og_t[:], in_=view(og, e0, p, k))

            gt_sl = [(i, j) for i, (p, k) in enumerate(PLAN) for j in range(k)]

            def gt_c(s):
                i, j = gt_sl[s]
                p, k = PLAN[i]
                c = 3 * (offs[i][1] + j)
                gts = pool.tile([p, SEG], F32, tag="gt", bufs=3,
                                padded_shape=[P, SEG])
                nc.vector.tensor_scalar_mul(gts[:], ones_t[:p, :],
                                            m_all[:p, c : c + 1])
                return gts

            def gt_s(s, gts):
                i, j = gt_sl[s]
                p, k = PLAN[i]
                gv = view(gt, offs[i][0], p, k)[:, j * SEG : (j + 1) * SEG]
                nc.scalar.dma_start(out=gv, in_=gts[:])

            def comp(i):
                p, k = PLAN[i]
                off = offs[i][1]
                a_tag, u_tag = ("as", "us") if i in SYNC_TILES else ("aa", "ua")
                at_t = pool.tile([p, k * SEG], F32, tag=a_tag, bufs=2,
                                 padded_shape=pad)
                uo_t = pool.tile([p, k * SEG], F32, tag=u_tag, bufs=2,
                                 padded_shape=pad)
                at_tiles[i], uo_tiles[i] = at_t, uo_t
                wm_t, og_t = in_tiles[i]
                for j in range(k):
                    sl = slice(j * SEG, (j + 1) * SEG)
                    c = 3 * (off + j)
                    s_am = m_all[:p, c + 0 : c + 1]  # 1 - attack
                    s_rm = m_all[:p, c + 1 : c + 2]  # revert
                    nc.vector.tensor_scalar_mul(at_t[:, sl], og_t[:, sl], s_rm)
                    nc.vector.scalar_tensor_tensor(
                        at_t[:, sl], wm_t[:, sl], s_am, at_t[:, sl], mult, add
                    )
                for j in range(k):
                    sl = slice(j * SEG, (j + 1) * SEG)
                    c = 3 * (off + j)
                    s_zm = m_all[:p, c + 2 : c + 3]  # 1 - zero
                    nc.vector.tensor_scalar_mul(uo_t[:, sl], og_t[:, sl], s_zm)

            def st_at(i, ring):
                p, k = PLAN[i]
                ring.dma_start(out=view(att, offs[i][0], p, k), in_=at_tiles[i][:])

            def st_uo(i, ring):
                p, k = PLAN[i]
                ring.dma_start(out=view(uo, offs[i][0], p, k), in_=uo_tiles[i][:])

            # Global DMA emission in chronological queue-need order: the
            # HWDGE completion-semaphore pool is ~10 deep and shared across
            # both queues, so DMA #n's issue waits for DMA #(n-10)'s
            # completion. Emitting in need order keeps every ring
            # predecessor ~25 us ahead. DVE compute ops are woven in just
            # before their first consuming store (writer-before-reader).
            nc.gpsimd.memset(ones_t[:], 1.0)
            nc.sync.dma_start(out=m_all[:], in_=mk[:, :])      # mask (sync q0)
            load(0)                                            # L0, L1
            g0 = gt_c(0); gt_s(0, g0)
            g1 = gt_c(1); gt_s(1, g1)
            load(1)                                            # L2, L3
            g2 = gt_c(2); gt_s(2, g2)
            g3 = gt_c(3); gt_s(3, g3)
            comp(0)                                            # t0 (sync tile)
            load(2)                                            # L4, L5
            g4 = gt_c(4); gt_s(4, g4)
            g5 = gt_c(5); gt_s(5, g5)
            g6 = gt_c(6); gt_s(6, g6)
            load(3)                                            # L6, L7
            g7 = gt_c(7); gt_s(7, g7)
            g8 = gt_c(8); gt_s(8, g8)
            g9 = gt_c(9); gt_s(9, g9)
            comp(1)
            st_at(1, nc.scalar)
            st_uo(1, nc.scalar)
            comp(2)
            st_at(2, nc.scalar)
            load(4)                                            # L8, L9
            st_uo(2, nc.scalar)
            comp(3)
            st_at(3, nc.scalar)
            st_uo(3, nc.scalar)
            load(5)                                            # L10, L11
            st_at(0, nc.sync)
            comp(4)
            st_at(4, nc.scalar)
            st_uo(0, nc.sync)
            comp(5)
            st_at(5, nc.sync)
            st_uo(4, nc.scalar)
            st_uo(5, nc.sync)
    nc.compile()
    return nc


_NC_CACHE: bass.Bass | None = None


def _pack_masks(oma_rows, rm_rows, omz_rows):
    """Per-core segment masks [N_SEGS] -> one [P, N_MASK_COLS] tile."""
    m_all = np.zeros((P, N_MASK_COLS), np.float32)
    r0 = 0
    off = 0
    for p, k in PLAN:
        for j in range(k):
            c = 3 * (off + j)
            # partition q, slice j holds segment r0 + q*k + j
            m_all[:p, c + 0] = oma_rows[r0 + j : r0 + p * k : k]
            m_all[:p, c + 1] = rm_rows[r0 + j : r0 + p * k : k]
            m_all[:p, c + 2] = omz_rows[r0 + j : r0 + p * k : k]
        r0 += p * k
        off += k
    return m_all


def _prepare_in_maps(original, watermarked, seg_starts, revert_flags):
    original = np.ascontiguousarray(np.asarray(original), dtype=np.float32)
    watermarked = np.ascontiguousarray(np.asarray(watermarked), dtype=np.float32)
    seg_starts = np.asarray(seg_starts)
    revert_flags = np.asarray(revert_flags)

    # Host-side segment masks, [B, 300] each (tiny).
    attack = np.zeros((B, S), np.float32)
    attack[np.arange(B)[:, None], seg_starts] = 1.0
    rf = revert_flags.astype(np.float32)
    one_minus_am = 1.0 - attack
    rm = attack * rf
    one_minus_zm = 1.0 - attack * (1.0 - rf)

    in_maps = []
    for c in range(N_CORES):
        sl = slice(c * B_LOC, (c + 1) * B_LOC)
        in_maps.append(
            {
                "wm": watermarked[sl].reshape(-1),
                "og": original[sl].reshape(-1),
                "mk": _pack_masks(
                    one_minus_am[sl].reshape(-1),
                    rm[sl].reshape(-1),
                    one_minus_zm[sl].reshape(-1),
                ),
            }
        )
    return in_maps


def _gather(results):
    def cat(name):
        return np.concatenate(
            [results[c][name].reshape(B_LOC, C, T) for c in range(N_CORES)], axis=0
        )

    return cat("att"), cat("gt"), cat("uo")


def _run(inputs: dict, **run_kwargs):
    global _NC_CACHE
    if _NC_CACHE is None:
        _NC_CACHE = _build_nc()
    in_maps = _prepare_in_maps(**inputs)
    res = run_bass_kernel_spmd(
        _NC_CACHE, in_maps, core_ids=list(range(N_CORES)), **run_kwargs
    )
    return res, _gather(res.results)


def kernel(original, watermarked, seg_starts, revert_flags):
    _, outs = _run(
        dict(
            original=original,
            watermarked=watermarked,
            seg_starts=seg_starts,
            revert_flags=revert_flags,
        )
    )
    return outs
